# revision 24
# baseline (speedup 1.0000x reference)
"""Context-only soft dot attention on 8 TRN2 NeuronCores, data-parallel over batch.

Per core (B_loc=32, S=2048, C=512, D=1024):
  target = h @ W.T                       (PE, one-time setup)
  logits[b,s] = <ctx[b,s,:], target[b]>  (DVE scalar_tensor_tensor: fused
                                          (ctx*mask01)*target_bcast + row-sum,
                                          mask folded in multiplicatively)
  p = exp(logits - 64)                   (ACT; fixed shift instead of max pass -
                                          masked logits become exp(-64)~=0)
  weighted[b] = sum_s p[s]*ctx[b,s,:]    (PE matvec, p column stationary,
                                          ctx tile moving, fp32r fast path)
  outputs normalized by 1/sum_s p at the end.

Single pass over context from HBM (~128 MiB/core) - DMA-bound by design.
"""

import sys
import numpy as np

sys.path.insert(0, "/opt/trn_rl_repo")

from contextlib import ExitStack

import concourse.bass as bass
import concourse.tile as tile
from concourse import bacc, mybir
from concourse.masks import make_identity

F32 = mybir.dt.float32
F32R = mybir.dt.float32r
U8 = mybir.dt.uint8
MULT = mybir.AluOpType.mult
ADD = mybir.AluOpType.add
IS_LT = mybir.AluOpType.is_lt
EXP = mybir.ActivationFunctionType.Exp
AX_X = mybir.AxisListType.X

B, S, DIM, CDIM = 256, 2048, 1024, 512
N_CORES = 8
B_LOC = B // N_CORES
EXP_SHIFT = -64.0


def build(b_loc=B_LOC, s=S, c=CDIM, d=DIM, mv2_dt=F32R):
    """Build the per-core Bass program. Returns nc."""
    nt = s // 128      # s-tiles per batch
    ncc = c // 128     # c chunks of 128
    ndc = d // 128     # d chunks of 128

    nc = bacc.Bacc(None)
    h_d = nc.declare_dram_parameter("h", [b_loc, d], F32, isOutput=False)
    ctx_d = nc.declare_dram_parameter("context", [b_loc, s, c], mv2_dt, isOutput=False)
    mask_d = nc.declare_dram_parameter("mask", [b_loc, s], U8, isOutput=False)
    w_d = nc.declare_dram_parameter("W", [c, d], F32, isOutput=False)
    outw_d = nc.declare_dram_parameter("out_weighted", [b_loc, c], F32, isOutput=True)
    outa_d = nc.declare_dram_parameter("out_attn", [b_loc, s], F32, isOutput=True)

    with tile.TileContext(nc) as tc, ExitStack() as ctx:
        singles = ctx.enter_context(tc.tile_pool(name="singles", bufs=1))
        ctx_pool = ctx.enter_context(tc.tile_pool(name="ctxp", bufs=3))
        wtmp_pool = ctx.enter_context(tc.tile_pool(name="wtmp", bufs=2))
        scratch = ctx.enter_context(tc.tile_pool(name="scratch", bufs=2))
        small = ctx.enter_context(tc.tile_pool(name="small", bufs=3))
        pone = ctx.enter_context(tc.tile_pool(name="pone", bufs=1, space="PSUM"))
        ptb = ctx.enter_context(tc.tile_pool(name="ptb", bufs=2, space="PSUM"))
        ptmp = ctx.enter_context(tc.tile_pool(name="ptmp", bufs=2, space="PSUM"))

        # ---- constants
        ident = singles.tile([128, 128], F32)
        make_identity(nc, ident)
        ones_r = singles.tile([1, 128], F32)
        nc.vector.memset(ones_r, 1.0)
        ones_c = singles.tile([128, 1], F32)
        nc.gpsimd.memset(ones_c, 1.0)
        shift_c = singles.tile([128, 1], F32)
        nc.gpsimd.memset(shift_c, EXP_SHIFT)

        # warm the exp table set early
        warm = small.tile([1, 1], F32)
        nc.gpsimd.memset(warm, 0.0)
        nc.scalar.activation(out=warm, in_=warm, func=EXP, bias=shift_c[0:1, :],
                             scale=1.0)

        # ---- mask01 = (mask < 1) as f32, then per-s-tile column layout
        mask_pool = ctx.enter_context(tc.tile_pool(name="maskp", bufs=1))
        mask_nat = mask_pool.tile([b_loc, s], U8, tag="masknat")
        nc.sync.dma_start(out=mask_nat, in_=mask_d[:])
        mask01 = mask_pool.tile([b_loc, s], F32, tag="mask01")
        nc.gpsimd.tensor_scalar(out=mask01, in0=mask_nat, scalar1=1, scalar2=None,
                                op0=IS_LT)
        mb = singles.tile([128, b_loc, nt], F32)   # mb[p, b, i] = mask01[b, 128i+p]
        for i in range(nt):
            pt = ptmp.tile([128, b_loc], F32, tag="tpose")
            nc.tensor.transpose(pt, mask01[:, i * 128:(i + 1) * 128],
                                ident[0:b_loc, 0:b_loc])
            nc.vector.tensor_copy(out=mb[:, :, i], in_=pt)

        # ---- W^T and h^T via PE transposes; target = h @ W.T  -> tgt [b_loc, c]
        wt = singles.tile([128, ndc, c], F32)      # wt[dp, k, c] = W[c, 128k+dp]
        for j in range(ncc):
            wn = wtmp_pool.tile([128, d], F32, tag="wn")
            nc.sync.dma_start(out=wn, in_=w_d[j * 128:(j + 1) * 128, :])
            for k in range(ndc):
                pt = ptmp.tile([128, 128], F32, tag="tpose")
                nc.tensor.transpose(pt, wn[:, k * 128:(k + 1) * 128], ident)
                nc.scalar.copy(out=wt[:, k, j * 128:(j + 1) * 128], in_=pt)
        hn = wtmp_pool.tile([b_loc, d], F32, tag="hn")
        nc.sync.dma_start(out=hn, in_=h_d[:])
        ht = singles.tile([128, ndc, b_loc], F32)  # ht[dp, k, b] = h[b, 128k+dp]
        for k in range(ndc):
            pt = ptmp.tile([128, b_loc], F32, tag="tpose")
            nc.tensor.transpose(pt, hn[:, k * 128:(k + 1) * 128],
                                ident[0:b_loc, 0:b_loc])
            nc.scalar.copy(out=ht[:, k, :], in_=pt)
        ptgt = pone.tile([b_loc, c], F32, tag="ptgt")
        for k in range(ndc):
            nc.tensor.matmul(ptgt, lhsT=ht[:, k, :], rhs=wt[:, k, :],
                             start=(k == 0), stop=(k == ndc - 1))
        tgt = singles.tile([b_loc, c], F32)
        nc.scalar.copy(out=tgt, in_=ptgt)

        # ---- accumulators
        wpart = singles.tile([b_loc, c], F32)     # unnormalized weighted rows
        sums_row = singles.tile([1, b_loc], F32)  # sum_s p per batch (part. 0)
        pbuf = singles.tile([128, b_loc, nt], mv2_dt)  # p = exp(logits-64)

        # ---- main loop over local batches
        for b in range(b_loc):
            ctx_t = ctx_pool.tile([128, nt, c], mv2_dt, tag="ctx")
            nc.sync.dma_start(out=ctx_t,
                              in_=ctx_d[b].rearrange("(i p) c -> p i c", p=128))
            # target row b to partition 0 (DMA can shift partitions; engines
            # cannot, and matmul operands need base_partition 0)
            trow = small.tile([1, c], F32, tag="trow")
            nc.sync.dma_start(out=trow, in_=tgt[b:b + 1, :])
            # broadcast target row b to 128 partitions (K=1 ones-matmul)
            tb = ptb.tile([128, c], F32, tag="tb")
            nc.tensor.matmul(tb, lhsT=ones_r, rhs=trow, start=True, stop=True)
            lb = small.tile([128, nt], F32, tag="lb")
            for i in range(nt):
                sc = scratch.tile([128, c], F32, tag="sc")
                nc.vector.scalar_tensor_tensor(
                    out=sc, in0=ctx_t[:, i, :], scalar=mb[:, b, i:i + 1],
                    in1=tb, op0=MULT, op1=MULT, accum_out=lb[:, i:i + 1])
            nc.scalar.activation(out=pbuf[:, b, :], in_=lb, func=EXP,
                                 bias=shift_c, scale=1.0)
            cs = small.tile([128, 1], F32, tag="cs")
            nc.vector.tensor_reduce(out=cs, in_=pbuf[:, b, :], axis=AX_X, op=ADD)
            psm = pone.tile([1, 2], F32, tag="psm")
            nc.tensor.matmul(psm[0:1, 0:1], lhsT=cs, rhs=ones_c,
                             start=True, stop=True)
            nc.scalar.copy(out=sums_row[0:1, b:b + 1], in_=psm[0:1, 0:1])
            pw = ptb.tile([1, c], F32, tag="pw")
            for i in range(nt):
                nc.tensor.matmul(pw, lhsT=pbuf[:, b, i:i + 1],
                                 rhs=ctx_t[:, i, :],
                                 start=(i == 0), stop=(i == nt - 1))
            wr = small.tile([1, c], F32, tag="wr")
            nc.scalar.copy(out=wr, in_=pw)
            nc.sync.dma_start(out=wpart[b:b + 1, :], in_=wr)

        # ---- epilogue: normalize
        rrow = small.tile([1, b_loc], F32, tag="rrow")
        nc.vector.reciprocal(rrow, sums_row)
        rec = small.tile([b_loc, 1], F32, tag="rec")
        nc.sync.dma_start(out=rec, in_=rrow)
        wsb = small.tile([b_loc, c], F32, tag="wsb")
        nc.vector.tensor_scalar(out=wsb, in0=wpart, scalar1=rec, scalar2=None,
                                op0=MULT)
        nc.sync.dma_start(out=outw_d[:], in_=wsb)

        # attn = pbuf * (1/sum) broadcast along partitions
        rrep = small.tile([1, b_loc, nt], F32, tag="rrep")
        rsrc = bass.AP(tensor=rrow.tensor, offset=rrow.offset,
                       ap=[rrow.ap[0], rrow.ap[1], [0, nt]])
        nc.vector.tensor_copy(out=rrep, in_=rsrc)
        rb = ptb.tile([128, b_loc, nt], F32, tag="tb")
        nc.tensor.matmul(rb, lhsT=ones_r, rhs=rrep, start=True, stop=True)
        attn_sb = singles.tile([128, b_loc, nt], F32)
        nc.vector.tensor_tensor(out=attn_sb, in0=pbuf, in1=rb, op=MULT)
        nc.sync.dma_start(out=outa_d[:].rearrange("b (i p) -> p b i", p=128),
                          in_=attn_sb)

    nc.finalize()
    return nc


_NC_CACHE = {}


def _get_nc():
    if "nc" not in _NC_CACHE:
        _NC_CACHE["nc"] = build()
    return _NC_CACHE["nc"]


def _run(inputs, trace=False, tmpdir=None):
    from concourse.bass_utils import run_bass_kernel_spmd

    nc = _get_nc()
    h = np.ascontiguousarray(np.asarray(inputs["h"], dtype=np.float32))
    context = np.ascontiguousarray(np.asarray(inputs["context"], dtype=np.float32))
    mask = np.ascontiguousarray(np.asarray(inputs["mask"])).view(np.uint8)
    w = np.ascontiguousarray(np.asarray(inputs["W"], dtype=np.float32))
    in_maps = []
    for i in range(N_CORES):
        sl = slice(i * B_LOC, (i + 1) * B_LOC)
        in_maps.append({
            "h": h[sl], "context": context[sl], "mask": mask[sl], "W": w,
        })
    res = run_bass_kernel_spmd(nc, in_maps, core_ids=list(range(N_CORES)),
                               trace=trace, tmpdir=tmpdir)
    weighted = np.concatenate([res.results[i]["out_weighted"]
                               for i in range(N_CORES)], axis=0)
    attn = np.concatenate([res.results[i]["out_attn"]
                           for i in range(N_CORES)], axis=0)
    return (weighted, attn), res


def kernel(**inputs):
    out, _ = _run(inputs, trace=False)
    return out


# revision 25
# speedup vs baseline: 1.0663x; 1.0663x over previous
"""Context-only soft dot attention on 8 TRN2 NeuronCores, data-parallel over batch.

Per core (B_loc=32, S=2048, C=512, D=1024):
  target = h @ W.T                       (PE, one-time setup)
  logits[b,s] = <ctx[b,s,:], target[b]>  (DVE scalar_tensor_tensor: fused
                                          (ctx*mask01)*target_bcast + row-sum,
                                          mask folded in multiplicatively)
  p = exp(logits - 64)                   (ACT; fixed shift instead of max pass -
                                          masked logits become exp(-64)~=0)
  weighted[b] = sum_s p[s]*ctx[b,s,:]    (PE matvec, p column stationary,
                                          ctx tile moving, fp32r fast path)
  outputs normalized by 1/sum_s p at the end.

Single pass over context from HBM (~128 MiB/core) - DMA-bound by design.
"""

import sys
import numpy as np

sys.path.insert(0, "/opt/trn_rl_repo")

from contextlib import ExitStack

import concourse.bass as bass
import concourse.tile as tile
from concourse import bacc, mybir
from concourse.masks import make_identity

F32 = mybir.dt.float32
F32R = mybir.dt.float32r
U8 = mybir.dt.uint8
MULT = mybir.AluOpType.mult
ADD = mybir.AluOpType.add
IS_LT = mybir.AluOpType.is_lt
EXP = mybir.ActivationFunctionType.Exp
AX_X = mybir.AxisListType.X

B, S, DIM, CDIM = 256, 2048, 1024, 512
N_CORES = 8
B_LOC = B // N_CORES
EXP_SHIFT = -64.0


def build(b_loc=B_LOC, s=S, c=CDIM, d=DIM, mv2_dt=F32R):
    """Build the per-core Bass program. Returns nc."""
    nt = s // 128      # s-tiles per batch
    ncc = c // 128     # c chunks of 128
    ndc = d // 128     # d chunks of 128

    nc = bacc.Bacc(None)
    h_d = nc.declare_dram_parameter("h", [b_loc, d], F32, isOutput=False)
    ctx_d = nc.declare_dram_parameter("context", [b_loc, s, c], mv2_dt, isOutput=False)
    mask_d = nc.declare_dram_parameter("mask", [b_loc, s], U8, isOutput=False)
    w_d = nc.declare_dram_parameter("W", [c, d], F32, isOutput=False)
    outw_d = nc.declare_dram_parameter("out_weighted", [b_loc, c], F32, isOutput=True)
    outa_d = nc.declare_dram_parameter("out_attn", [b_loc, s], F32, isOutput=True)

    with tile.TileContext(nc) as tc, ExitStack() as ctx:
        singles = ctx.enter_context(tc.tile_pool(name="singles", bufs=1))
        ctx_pool = ctx.enter_context(tc.tile_pool(name="ctxp", bufs=3))
        wtmp_pool = ctx.enter_context(tc.tile_pool(name="wtmp", bufs=2))
        scratch = ctx.enter_context(tc.tile_pool(name="scratch", bufs=3))
        small = ctx.enter_context(tc.tile_pool(name="small", bufs=4))
        pone = ctx.enter_context(tc.tile_pool(name="pone", bufs=1, space="PSUM"))
        ptb = ctx.enter_context(tc.tile_pool(name="ptb", bufs=2, space="PSUM"))
        ptmp = ctx.enter_context(tc.tile_pool(name="ptmp", bufs=2, space="PSUM"))

        # ---- constants
        ident = singles.tile([128, 128], F32)
        make_identity(nc, ident)
        ones_f = singles.tile([1, 128], F32)
        nc.vector.memset(ones_f, 1.0)
        ones_r = singles.tile([1, 128], F32R)
        nc.scalar.copy(out=ones_r, in_=ones_f)
        ones_c = singles.tile([128, 1], F32)
        nc.gpsimd.memset(ones_c, 1.0)
        shift_c = singles.tile([128, 1], F32)
        nc.gpsimd.memset(shift_c, EXP_SHIFT)

        # warm the exp table set early
        warm = small.tile([1, 1], F32)
        nc.gpsimd.memset(warm, 0.0)
        nc.scalar.activation(out=warm, in_=warm, func=EXP, bias=shift_c[0:1, :],
                             scale=1.0)

        # ---- mask01 = (mask < 1) as f32, then per-s-tile column layout
        mask_pool = ctx.enter_context(tc.tile_pool(name="maskp", bufs=1))
        mask_nat = mask_pool.tile([b_loc, s], U8, tag="masknat")
        nc.sync.dma_start(out=mask_nat, in_=mask_d[:])
        mask01 = mask_pool.tile([b_loc, s], F32, tag="mask01")
        nc.gpsimd.tensor_scalar(out=mask01, in0=mask_nat, scalar1=1, scalar2=None,
                                op0=IS_LT)
        mb = singles.tile([128, b_loc, nt], F32)   # mb[p, b, i] = mask01[b, 128i+p]
        for i in range(nt):
            pt = ptmp.tile([128, b_loc], F32, tag="tpose")
            nc.tensor.transpose(pt, mask01[:, i * 128:(i + 1) * 128],
                                ident[0:b_loc, 0:b_loc])
            nc.vector.tensor_copy(out=mb[:, :, i], in_=pt)

        # ---- W^T and h^T via PE transposes; target = h @ W.T  -> tgt [b_loc, c]
        wt = singles.tile([128, ndc, c], F32)      # wt[dp, k, c] = W[c, 128k+dp]
        for j in range(ncc):
            wn = wtmp_pool.tile([128, d], F32, tag="wn")
            nc.sync.dma_start(out=wn, in_=w_d[j * 128:(j + 1) * 128, :])
            for k in range(ndc):
                pt = ptmp.tile([128, 128], F32, tag="tpose")
                nc.tensor.transpose(pt, wn[:, k * 128:(k + 1) * 128], ident)
                nc.scalar.copy(out=wt[:, k, j * 128:(j + 1) * 128], in_=pt)
        hn = wtmp_pool.tile([b_loc, d], F32, tag="hn")
        nc.sync.dma_start(out=hn, in_=h_d[:])
        ht = singles.tile([128, ndc, b_loc], F32)  # ht[dp, k, b] = h[b, 128k+dp]
        for k in range(ndc):
            pt = ptmp.tile([128, b_loc], F32, tag="tpose")
            nc.tensor.transpose(pt, hn[:, k * 128:(k + 1) * 128],
                                ident[0:b_loc, 0:b_loc])
            nc.scalar.copy(out=ht[:, k, :], in_=pt)
        ptgt = pone.tile([b_loc, c], F32, tag="ptgt")
        for k in range(ndc):
            nc.tensor.matmul(ptgt, lhsT=ht[:, k, :], rhs=wt[:, k, :],
                             start=(k == 0), stop=(k == ndc - 1))
        tgt = singles.tile([b_loc, c], F32R)
        nc.scalar.copy(out=tgt, in_=ptgt)

        # ---- accumulators
        wpart = singles.tile([b_loc, c], F32)     # unnormalized weighted rows
        sums_row = singles.tile([1, b_loc], F32)  # sum_s p per batch (part. 0)
        pbuf = singles.tile([128, b_loc, nt], mv2_dt)  # p = exp(logits-64)

        # ---- main loop over local batches
        for b in range(b_loc):
            ctx_t = ctx_pool.tile([128, nt, c], mv2_dt, tag="ctx")
            nc.sync.dma_start(out=ctx_t,
                              in_=ctx_d[b].rearrange("(i p) c -> p i c", p=128))
            # target row b to partition 0 (DMA can shift partitions; engines
            # cannot, and matmul operands need base_partition 0)
            trow = small.tile([1, c], F32R, tag="trow")
            nc.sync.dma_start(out=trow, in_=tgt[b:b + 1, :])
            # broadcast target row b to 128 partitions (K=1 ones-matmul)
            tb = ptb.tile([128, c], F32, tag="tb")
            nc.tensor.matmul(tb, lhsT=ones_r, rhs=trow, start=True, stop=True)
            lb = small.tile([128, nt], F32, tag="lb")
            for i in range(nt):
                sc = scratch.tile([128, c], F32, tag="sc")
                nc.vector.scalar_tensor_tensor(
                    out=sc, in0=ctx_t[:, i, :], scalar=mb[:, b, i:i + 1],
                    in1=tb, op0=MULT, op1=MULT, accum_out=lb[:, i:i + 1])
            nc.scalar.activation(out=pbuf[:, b, :], in_=lb, func=EXP,
                                 bias=shift_c, scale=1.0)
            cs = small.tile([128, 1], F32, tag="cs")
            nc.vector.tensor_reduce(out=cs, in_=pbuf[:, b, :], axis=AX_X, op=ADD)
            psm = pone.tile([1, 2], F32, tag="psm")
            nc.tensor.matmul(psm[0:1, 0:1], lhsT=cs, rhs=ones_c,
                             start=True, stop=True)
            nc.scalar.copy(out=sums_row[0:1, b:b + 1], in_=psm[0:1, 0:1])
            pw = ptb.tile([1, c], F32, tag="pw")
            for i in range(nt):
                nc.tensor.matmul(pw, lhsT=pbuf[:, b, i:i + 1],
                                 rhs=ctx_t[:, i, :],
                                 start=(i == 0), stop=(i == nt - 1))
            wr = small.tile([1, c], F32, tag="wr")
            nc.scalar.copy(out=wr, in_=pw)
            nc.sync.dma_start(out=wpart[b:b + 1, :], in_=wr)

        # ---- epilogue: normalize
        rrow = small.tile([1, b_loc], F32, tag="rrow")
        nc.vector.reciprocal(rrow, sums_row)
        rec = small.tile([b_loc, 1], F32, tag="rec")
        nc.sync.dma_start(out=rec, in_=rrow)
        wsb = small.tile([b_loc, c], F32, tag="wsb")
        nc.vector.tensor_scalar(out=wsb, in0=wpart, scalar1=rec, scalar2=None,
                                op0=MULT)
        nc.sync.dma_start(out=outw_d[:], in_=wsb)

        # attn = pbuf * (1/sum) broadcast along partitions
        rrep = small.tile([1, b_loc, nt], F32, tag="rrep")
        rsrc = bass.AP(tensor=rrow.tensor, offset=rrow.offset,
                       ap=[rrow.ap[0], rrow.ap[1], [0, nt]])
        nc.vector.tensor_copy(out=rrep, in_=rsrc)
        rb = ptb.tile([128, b_loc, nt], F32, tag="tb")
        nc.tensor.matmul(rb, lhsT=ones_f, rhs=rrep, start=True, stop=True)
        attn_sb = singles.tile([128, b_loc, nt], F32)
        nc.vector.tensor_tensor(out=attn_sb, in0=pbuf, in1=rb, op=MULT)
        nc.sync.dma_start(out=outa_d[:].rearrange("b (i p) -> p b i", p=128),
                          in_=attn_sb)

    nc.finalize()
    return nc


_NC_CACHE = {}


def _get_nc():
    if "nc" not in _NC_CACHE:
        _NC_CACHE["nc"] = build()
    return _NC_CACHE["nc"]


def _run(inputs, trace=False, tmpdir=None):
    from concourse.bass_utils import run_bass_kernel_spmd

    nc = _get_nc()
    h = np.ascontiguousarray(np.asarray(inputs["h"], dtype=np.float32))
    context = np.ascontiguousarray(np.asarray(inputs["context"], dtype=np.float32))
    mask = np.ascontiguousarray(np.asarray(inputs["mask"])).view(np.uint8)
    w = np.ascontiguousarray(np.asarray(inputs["W"], dtype=np.float32))
    in_maps = []
    for i in range(N_CORES):
        sl = slice(i * B_LOC, (i + 1) * B_LOC)
        in_maps.append({
            "h": h[sl], "context": context[sl], "mask": mask[sl], "W": w,
        })
    res = run_bass_kernel_spmd(nc, in_maps, core_ids=list(range(N_CORES)),
                               trace=trace, tmpdir=tmpdir)
    weighted = np.concatenate([res.results[i]["out_weighted"]
                               for i in range(N_CORES)], axis=0)
    attn = np.concatenate([res.results[i]["out_attn"]
                           for i in range(N_CORES)], axis=0)
    return (weighted, attn), res


def kernel(**inputs):
    out, _ = _run(inputs, trace=False)
    return out


# revision 26
# speedup vs baseline: 1.1040x; 1.0353x over previous
"""Context-only soft dot attention on 8 TRN2 NeuronCores, data-parallel over batch.

Per core (B_loc=32, S=2048, C=512, D=1024):
  target = h @ W.T                       (PE, one-time setup)
  logits[b,s] = <ctx[b,s,:], target[b]>  (DVE scalar_tensor_tensor: fused
                                          (ctx*mask01)*target_bcast + row-sum,
                                          mask folded in multiplicatively)
  p = exp(logits - 64)                   (ACT; fixed shift instead of max pass -
                                          masked logits become exp(-64)~=0)
  weighted[b] = sum_s p[s]*ctx[b,s,:]    (PE matvec, p column stationary,
                                          ctx tile moving, fp32r fast path)
  outputs normalized by 1/sum_s p at the end.

Single pass over context from HBM (~128 MiB/core) - DMA-bound by design.
"""

import sys
import numpy as np

sys.path.insert(0, "/opt/trn_rl_repo")

from contextlib import ExitStack

import concourse.bass as bass
import concourse.tile as tile
from concourse import bacc, mybir
from concourse.masks import make_identity

F32 = mybir.dt.float32
F32R = mybir.dt.float32r
U8 = mybir.dt.uint8
MULT = mybir.AluOpType.mult
ADD = mybir.AluOpType.add
IS_LT = mybir.AluOpType.is_lt
EXP = mybir.ActivationFunctionType.Exp
AX_X = mybir.AxisListType.X

B, S, DIM, CDIM = 256, 2048, 1024, 512
N_CORES = 8
B_LOC = B // N_CORES
EXP_SHIFT = -64.0


def build(b_loc=B_LOC, s=S, c=CDIM, d=DIM, mv2_dt=F32R):
    """Build the per-core Bass program. Returns nc."""
    nt = s // 128      # s-tiles per batch
    ncc = c // 128     # c chunks of 128
    ndc = d // 128     # d chunks of 128

    nc = bacc.Bacc(None)
    h_d = nc.declare_dram_parameter("h", [b_loc, d], F32, isOutput=False)
    ctx_d = nc.declare_dram_parameter("context", [b_loc, s, c], mv2_dt, isOutput=False)
    mask_d = nc.declare_dram_parameter("mask", [b_loc, s], U8, isOutput=False)
    w_d = nc.declare_dram_parameter("W", [c, d], F32, isOutput=False)
    outw_d = nc.declare_dram_parameter("out_weighted", [b_loc, c], F32, isOutput=True)
    outa_d = nc.declare_dram_parameter("out_attn", [b_loc, s], F32, isOutput=True)

    with tile.TileContext(nc) as tc, ExitStack() as ctx:
        singles = ctx.enter_context(tc.tile_pool(name="singles", bufs=1))
        ctx_pool = ctx.enter_context(tc.tile_pool(name="ctxp", bufs=3))
        wtmp_pool = ctx.enter_context(tc.tile_pool(name="wtmp", bufs=2))
        scratch = ctx.enter_context(tc.tile_pool(name="scratch", bufs=3))
        small = ctx.enter_context(tc.tile_pool(name="small", bufs=4))
        pone = ctx.enter_context(tc.tile_pool(name="pone", bufs=1, space="PSUM"))
        ptb = ctx.enter_context(tc.tile_pool(name="ptb", bufs=2, space="PSUM"))
        ptmp = ctx.enter_context(tc.tile_pool(name="ptmp", bufs=2, space="PSUM"))

        # ---- constants
        ident = singles.tile([128, 128], F32)
        make_identity(nc, ident)
        ones_f = singles.tile([1, 128], F32)
        nc.vector.memset(ones_f, 1.0)
        ones_r = singles.tile([1, 128], F32R)
        nc.scalar.copy(out=ones_r, in_=ones_f)
        ones_c = singles.tile([128, 1], F32)
        nc.gpsimd.memset(ones_c, 1.0)
        shift_c = singles.tile([128, 1], F32)
        nc.gpsimd.memset(shift_c, EXP_SHIFT)

        # warm the exp table set early
        warm = small.tile([1, 1], F32)
        nc.gpsimd.memset(warm, 0.0)
        nc.scalar.activation(out=warm, in_=warm, func=EXP, bias=shift_c[0:1, :],
                             scale=1.0)

        # ---- mask01 = (mask < 1) as f32, then per-s-tile column layout
        mask_pool = ctx.enter_context(tc.tile_pool(name="maskp", bufs=1))
        mask_nat = mask_pool.tile([b_loc, s], U8, tag="masknat")
        nc.sync.dma_start(out=mask_nat, in_=mask_d[:])
        mask01 = mask_pool.tile([b_loc, s], F32, tag="mask01")
        nc.gpsimd.tensor_scalar(out=mask01, in0=mask_nat, scalar1=1, scalar2=None,
                                op0=IS_LT)
        mb = singles.tile([128, b_loc, nt], F32)   # mb[p, b, i] = mask01[b, 128i+p]
        for i in range(nt):
            pt = ptmp.tile([128, b_loc], F32, tag="tpose")
            nc.tensor.transpose(pt, mask01[:, i * 128:(i + 1) * 128],
                                ident[0:b_loc, 0:b_loc])
            nc.vector.tensor_copy(out=mb[:, :, i], in_=pt)

        # ---- W^T and h^T via PE transposes; target = h @ W.T  -> tgt [b_loc, c]
        wt = singles.tile([128, ndc, c], F32)      # wt[dp, k, c] = W[c, 128k+dp]
        for j in range(ncc):
            wn = wtmp_pool.tile([128, d], F32, tag="wn")
            nc.sync.dma_start(out=wn, in_=w_d[j * 128:(j + 1) * 128, :])
            for k in range(ndc):
                pt = ptmp.tile([128, 128], F32, tag="tpose")
                nc.tensor.transpose(pt, wn[:, k * 128:(k + 1) * 128], ident)
                nc.scalar.copy(out=wt[:, k, j * 128:(j + 1) * 128], in_=pt)
        hn = wtmp_pool.tile([b_loc, d], F32, tag="hn")
        nc.sync.dma_start(out=hn, in_=h_d[:])
        ht = singles.tile([128, ndc, b_loc], F32)  # ht[dp, k, b] = h[b, 128k+dp]
        for k in range(ndc):
            pt = ptmp.tile([128, b_loc], F32, tag="tpose")
            nc.tensor.transpose(pt, hn[:, k * 128:(k + 1) * 128],
                                ident[0:b_loc, 0:b_loc])
            nc.scalar.copy(out=ht[:, k, :], in_=pt)
        ptgt = pone.tile([b_loc, c], F32, tag="ptgt")
        for k in range(ndc):
            nc.tensor.matmul(ptgt, lhsT=ht[:, k, :], rhs=wt[:, k, :],
                             start=(k == 0), stop=(k == ndc - 1))
        tgt = singles.tile([b_loc, c], F32R)
        nc.scalar.copy(out=tgt, in_=ptgt)

        # ---- accumulators
        wpart = singles.tile([b_loc, c], F32)     # unnormalized weighted rows
        sums_row = singles.tile([1, b_loc], F32)  # sum_s p per batch (part. 0)
        pbuf = singles.tile([128, b_loc, nt], mv2_dt)  # p = exp(logits-64)

        # ---- main loop over local batches
        for b in range(b_loc):
            ctx_t = ctx_pool.tile([128, nt, c], mv2_dt, tag="ctx")
            h1 = nt // 2
            src_ap = ctx_d[b].rearrange("(i p) c -> p i c", p=128)
            nc.sync.dma_start(out=ctx_t[:, 0:h1, :], in_=src_ap[:, 0:h1, :])
            nc.sync.dma_start(out=ctx_t[:, h1:nt, :], in_=src_ap[:, h1:nt, :])
            # target row b to partition 0 (DMA can shift partitions; engines
            # cannot, and matmul operands need base_partition 0)
            trow = small.tile([1, c], F32R, tag="trow")
            nc.sync.dma_start(out=trow, in_=tgt[b:b + 1, :])
            # broadcast target row b to 128 partitions (K=1 ones-matmul)
            tb = ptb.tile([128, c], F32, tag="tb")
            nc.tensor.matmul(tb, lhsT=ones_r, rhs=trow, start=True, stop=True)
            lb = small.tile([128, nt], F32, tag="lb")
            for i in range(nt):
                sc = scratch.tile([128, c], F32, tag="sc")
                nc.vector.scalar_tensor_tensor(
                    out=sc, in0=ctx_t[:, i, :], scalar=mb[:, b, i:i + 1],
                    in1=tb, op0=MULT, op1=MULT, accum_out=lb[:, i:i + 1])
            nc.scalar.activation(out=pbuf[:, b, :], in_=lb, func=EXP,
                                 bias=shift_c, scale=1.0)
            cs = small.tile([128, 1], F32, tag="cs")
            nc.vector.tensor_reduce(out=cs, in_=pbuf[:, b, :], axis=AX_X, op=ADD)
            psm = pone.tile([1, 2], F32, tag="psm")
            nc.tensor.matmul(psm[0:1, 0:1], lhsT=cs, rhs=ones_c,
                             start=True, stop=True)
            nc.scalar.copy(out=sums_row[0:1, b:b + 1], in_=psm[0:1, 0:1])
            pw = ptb.tile([1, c], F32, tag="pw")
            for i in range(nt):
                nc.tensor.matmul(pw, lhsT=pbuf[:, b, i:i + 1],
                                 rhs=ctx_t[:, i, :],
                                 start=(i == 0), stop=(i == nt - 1))
            wr = small.tile([1, c], F32, tag="wr")
            nc.scalar.copy(out=wr, in_=pw)
            nc.sync.dma_start(out=wpart[b:b + 1, :], in_=wr)

        # ---- epilogue: normalize
        rrow = small.tile([1, b_loc], F32, tag="rrow")
        nc.vector.reciprocal(rrow, sums_row)
        rec = small.tile([b_loc, 1], F32, tag="rec")
        nc.sync.dma_start(out=rec, in_=rrow)
        wsb = small.tile([b_loc, c], F32, tag="wsb")
        nc.vector.tensor_scalar(out=wsb, in0=wpart, scalar1=rec, scalar2=None,
                                op0=MULT)
        nc.sync.dma_start(out=outw_d[:], in_=wsb)

        # attn = pbuf * (1/sum) broadcast along partitions
        rrep = small.tile([1, b_loc, nt], F32, tag="rrep")
        rsrc = bass.AP(tensor=rrow.tensor, offset=rrow.offset,
                       ap=[rrow.ap[0], rrow.ap[1], [0, nt]])
        nc.vector.tensor_copy(out=rrep, in_=rsrc)
        rb = ptb.tile([128, b_loc, nt], F32, tag="tb")
        nc.tensor.matmul(rb, lhsT=ones_f, rhs=rrep, start=True, stop=True)
        attn_sb = singles.tile([128, b_loc, nt], F32)
        nc.vector.tensor_tensor(out=attn_sb, in0=pbuf, in1=rb, op=MULT)
        nc.sync.dma_start(out=outa_d[:].rearrange("b (i p) -> p b i", p=128),
                          in_=attn_sb)

    nc.finalize()
    return nc


_NC_CACHE = {}


def _get_nc():
    if "nc" not in _NC_CACHE:
        _NC_CACHE["nc"] = build()
    return _NC_CACHE["nc"]


def _run(inputs, trace=False, tmpdir=None):
    from concourse.bass_utils import run_bass_kernel_spmd

    nc = _get_nc()
    h = np.ascontiguousarray(np.asarray(inputs["h"], dtype=np.float32))
    context = np.ascontiguousarray(np.asarray(inputs["context"], dtype=np.float32))
    mask = np.ascontiguousarray(np.asarray(inputs["mask"])).view(np.uint8)
    w = np.ascontiguousarray(np.asarray(inputs["W"], dtype=np.float32))
    in_maps = []
    for i in range(N_CORES):
        sl = slice(i * B_LOC, (i + 1) * B_LOC)
        in_maps.append({
            "h": h[sl], "context": context[sl], "mask": mask[sl], "W": w,
        })
    res = run_bass_kernel_spmd(nc, in_maps, core_ids=list(range(N_CORES)),
                               trace=trace, tmpdir=tmpdir)
    weighted = np.concatenate([res.results[i]["out_weighted"]
                               for i in range(N_CORES)], axis=0)
    attn = np.concatenate([res.results[i]["out_attn"]
                           for i in range(N_CORES)], axis=0)
    return (weighted, attn), res


def kernel(**inputs):
    out, _ = _run(inputs, trace=False)
    return out
